# revision 1
# baseline (speedup 1.0000x reference)
"""Complex-valued multi-head attention on 8 Trainium2 NeuronCores.

Sharding: batch(2) x head-pairs(4) -> 8 cores; each core runs one batch
element and 2 heads end-to-end (QKV proj -> complex scores -> |s| softmax
-> AV -> partial W_O), host sums the W_O partials over the 4 cores of each
batch element (tensor-parallel reduce) and transposes to the output layout.

Inputs, weights and all matmuls run in fp16 (psum accumulation is fp32);
projections emit per-head stacked [re;im] tensors so each complex score
matrix is a single K=128 matmul (host packs the stacked weights). Softmax
uses exp(|s|/8 - 1.5) without max-subtraction (|s| is bounded small), the
rowsum rides a ones-column matmul on the transposed probabilities, and the
1/rowsum normalization is applied to the 64-wide AV output instead of the
2048-wide attention matrix. sqrt and exp are batched per q-chunk with
explicit ACT table-set loads (sqrt/exp live in different table sets), and
AV/normalize/W_O for q-chunk i are emitted during q-chunk i+1 so PE never
stalls on the softmax chain at chunk boundaries.
"""
import sys

sys.path.insert(0, "/opt/trn_rl_repo")

import numpy as np

B, NQ, NK, R = 2, 2048, 2048, 512
H, DK, DV = 8, 64, 64
NCORES = 8
NCC = 8          # n-chunks for projection streaming (2048/256)
NCW = 256        # projection n-chunk width
QC = 4           # q-chunks in attention (2048/512)
QCW = 512
KT = 16          # k-tiles (2048/128)

_CACHE = {}
DEBUG = False


def _build_nc():
    import concourse.bass as bass
    import concourse.tile as tile
    from concourse.tile import add_dep_helper
    from concourse import bacc, mybir

    f32 = mybir.dt.float32
    f32r = mybir.dt.float32r
    f16 = mybir.dt.float16
    ALU = mybir.AluOpType
    AF = mybir.ActivationFunctionType

    nc = bacc.Bacc("TRN2", target_bir_lowering=False, debug=False,
                   num_devices=NCORES)

    xpack_e = nc.dram_tensor("xpack", [NCC, 24, 128, NCW], f16,
                             kind="ExternalInput")
    wpack_e = nc.dram_tensor("wpack", [128, 64 * 128], f16,
                             kind="ExternalInput")
    wopack_e = nc.dram_tensor("wopack", [128, 3 * 512], f16,
                              kind="ExternalInput")
    onesr_e = nc.dram_tensor("onesr", [1, 128], f32r, kind="ExternalInput")
    ident_e = nc.dram_tensor("ident", [128, 128], f16, kind="ExternalInput")
    dbg_es = {}
    if DEBUG:
        for nm, w in (("d_v16_h0", NK), ("d_oT_re", NQ),
                      ("d_p00", 1024), ("d_rs0", 512)):
            dbg_es[nm] = nc.dram_tensor(nm, [128, w], f32,
                                        kind="ExternalOutput")
    ore_e = nc.dram_tensor("out_re", [512, NQ], f32, kind="ExternalOutput")
    oim_e = nc.dram_tensor("out_im", [512, NQ], f32, kind="ExternalOutput")

    with tile.TileContext(nc) as tc:
      with nc.allow_low_precision(reason="fp16 softmax path"):
        with tc.tile_pool(name="pers", bufs=1) as pers, \
             tc.tile_pool(name="work", bufs=2) as work, \
             tc.tile_pool(name="pwork", bufs=3) as pwork, \
             tc.tile_pool(name="psA", bufs=1, space="PSUM") as psA:

            # Preload the one ACT table set that covers square/ln/exp/copy
            # so the table-load pass never needs to thrash sets.
            from concourse.hw_specs import get_activation_tables
            tables = list(get_activation_tables(nc.m.arch).keys())
            SQRT_SET = tables.index("sqrt_and_others")
            EXP_SET = tables.index("exp_and_others")

            def load_act_set(set_id):
                lafs = mybir.InstLoadActFuncSet(
                    name=nc.get_next_instruction_name(), ins=[], outs=[],
                    act_func_set_id=set_id)
                lafs.engine = mybir.EngineType.Activation
                nc.scalar.add_instruction(lafs)

            load_act_set(EXP_SET)

            # ---- constants ----
            wp = pers.tile([128, 64 * 128], f16, tag="wp")
            nc.sync.dma_start(wp[:], wpack_e[:])
            wop = pers.tile([128, 3 * 512], f16, tag="wop")
            nc.sync.dma_start(wop[:], wopack_e[:])
            ones_row = pers.tile([1, 128], f32r, tag="ones_row")
            nc.sync.dma_start(ones_row[:], onesr_e[:])
            ident16 = pers.tile([128, 128], f16, tag="ident16")
            nc.sync.dma_start(ident16[:], ident_e[:])
            ones16 = pers.tile([128, 1], f16, tag="ones16")
            nc.vector.memset(ones16[:], 1.0)
            eb_exp = pers.tile([128, 1], f32, tag="eb_exp")
            nc.vector.memset(eb_exp[:], -1.5)          # exp(mag - 1.5)

            # ---- projections -> per-head stacked [re;im] tensors fp16 ----
            # q_sb[h] = [q_r_h; q_i_h], A_sb[h] = [k_r_h; k_i_h],
            # C_sb[h] = [-k_i_h; k_r_h], vT16_h[h] = [v_r_h; v_i_h]
            q_sb = [pers.tile([128, NQ], f16, tag=f"q_sb{h}",
                              name=f"q_sb{h}") for h in (0, 1)]
            A_sb = [pers.tile([128, NK], f16, tag=f"A_sb{h}",
                              name=f"A_sb{h}") for h in (0, 1)]
            C_sb = [pers.tile([128, NK], f16, tag=f"C_sb{h}",
                              name=f"C_sb{h}") for h in (0, 1)]
            vT16_h = [pers.tile([128, NK], f16, tag=f"vT16_h{h}",
                                name=f"vT16_h{h}") for h in (0, 1)]

            # dest i uses weight blocks w=2i (for x_re) and w=2i+1 (for x_im)
            # x-block index: t=0..1 q_re/q_im, 2..3 k, 4..5 v
            specs = [
                (q_sb[0], 0), (q_sb[1], 0), (A_sb[0], 2), (A_sb[1], 2),
                (C_sb[0], 2), (C_sb[1], 2), (vT16_h[0], 4), (vT16_h[1], 4),
            ]
            v16_h = [pers.tile([128, NK], f16, tag=f"v16_h{h}",
                               name=f"v16_h{h}") for h in (0, 1)]
            for ncc in range(NCC):
                xt = work.tile([128, 24 * NCW], f16, tag="xt")
                nc.sync.dma_start(
                    xt[:].rearrange("p (b f) -> p b f", f=NCW),
                    xpack_e[ncc].rearrange("b p f -> p b f"))

                def xblk(t, rc):
                    return xt[:, (t * 4 + rc) * NCW:(t * 4 + rc + 1) * NCW]

                def wblk(w, rc):
                    return wp[:, (w * 4 + rc) * 128:(w * 4 + rc + 1) * 128]

                for si, (dest, tx) in enumerate(specs):
                    pj = psA.tile([128, NCW], f32,
                                  tag=("s_re" if si % 2 == 0 else "s_im"),
                                  name=f"pj_{ncc}_{si}")
                    for rc in range(4):
                        nc.tensor.matmul(pj[:], wblk(2 * si, rc),
                                         xblk(tx, rc),
                                         start=(rc == 0), stop=False)
                    for rc in range(4):
                        nc.tensor.matmul(pj[:], wblk(2 * si + 1, rc),
                                         xblk(tx + 1, rc),
                                         start=False, stop=(rc == 3))
                    cs = slice(ncc * NCW, (ncc + 1) * NCW)
                    if si % 2 == 0:
                        nc.scalar.copy(dest[:, cs], pj[:])
                    else:
                        nc.vector.tensor_copy(dest[:, cs], pj[:])
                # transpose this ncc's V columns right away
                for h in (0, 1):
                    for nt in (2 * ncc, 2 * ncc + 1):
                        blk = slice(nt * 128, (nt + 1) * 128)
                        vt_ps = psA.tile([128, 128], f16, tag="s_im",
                                         name=f"vtp_{h}_{nt}")
                        nc.tensor.transpose(vt_ps[:], vT16_h[h][:, blk],
                                            ident16[:])
                        if (h + nt) % 2 == 0:
                            nc.vector.tensor_copy(v16_h[h][:, blk], vt_ps[:])
                        else:
                            nc.scalar.copy(v16_h[h][:, blk], vt_ps[:])

            # ---- output accumulators for W_O ----
            oT_re = pers.tile([128, NQ], f16, tag="oT_re")
            oT_im = pers.tile([128, NQ], f16, tag="oT_im")

            # ---- attention ----
            def emit_wo(qs, qcn):
                for Rc in range(4):
                    wo_re = psA.tile([128, QCW], f32, tag="s_re",
                                     name=f"wore_{Rc}_{qcn}")
                    wo_im = psA.tile([128, QCW], f32, tag="s_im",
                                     name=f"woim_{Rc}_{qcn}")

                    def wob(w):
                        return wop[:, w * 512 + Rc * 128:
                                   w * 512 + Rc * 128 + 128]

                    nc.tensor.matmul(wo_re[:], wob(0), oT_re[:, qs],
                                     start=True, stop=False)
                    nc.tensor.matmul(wo_re[:], wob(2), oT_im[:, qs],
                                     start=False, stop=True)
                    nc.tensor.matmul(wo_im[:], wob(1), oT_re[:, qs],
                                     start=True, stop=False)
                    nc.tensor.matmul(wo_im[:], wob(0), oT_im[:, qs],
                                     start=False, stop=True)
                    st_re = work.tile([128, QCW], f32, tag="st_re")
                    nc.vector.tensor_copy(st_re[:], wo_re[:])
                    nc.sync.dma_start(
                        ore_e[Rc * 128:(Rc + 1) * 128, qs], st_re[:])
                    st_im = work.tile([128, QCW], f32, tag="st_im")
                    nc.vector.tensor_copy(st_im[:], wo_im[:])
                    nc.sync.dma_start(
                        oim_e[Rc * 128:(Rc + 1) * 128, qs], st_im[:])

            def emit_norm(o_ps, rs_ps, qs, qcn):
                bc = psA.tile([128, 1024], f32, tag="s_re",
                              name=f"bc_{qcn}")
                for h in (0, 1):
                    recip = work.tile([1, QCW], f32r, tag=f"recip{h}")
                    nc.vector.reciprocal(recip[:], rs_ps[h][0:1, :])
                    nc.tensor.matmul(bc[:, h * 512:h * 512 + 512],
                                     ones_row[:], recip[:],
                                     start=True, stop=True)
                bc_sb = work.tile([128, 1024], f32r, tag="bc_sb")
                nc.scalar.copy(bc_sb[:], bc[:])
                for h in (0, 1):
                    for ri, dest in ((0, oT_re), (1, oT_im)):
                        rows = slice(64 * ri, 64 * ri + 64)
                        nc.vector.scalar_tensor_tensor(
                            dest[64 * h:64 * h + 64, qs],
                            o_ps[h][rows, :], 1.0,
                            bc_sb[rows, h * 512:h * 512 + 512],
                            ALU.mult, ALU.mult)

            NB = 2  # transcendental batches per q-chunk

            def emit_av(pbs, qs, qcn):
                o_ps = [psA.tile([128, QCW], f32, tag=f"o{h}",
                                 name=f"o{h}_{qcn}") for h in (0, 1)]
                rs_ps = [psA.tile([128, QCW], f32, tag=f"rs{h}",
                                  name=f"rs{h}_{qcn}") for h in (0, 1)]
                HB = KT // NB
                for half in range(NB):
                    for k8 in range(HB):
                        kt = half * HB + k8
                        for h in (0, 1):
                            col = slice(k8 * 1024 + h * 512,
                                        k8 * 1024 + h * 512 + 512)
                            vblk = v16_h[h][:, kt * 128:(kt + 1) * 128]
                            nc.tensor.matmul(o_ps[h][:, :], vblk,
                                             pbs[half][:, col],
                                             start=(kt == 0),
                                             stop=(kt == KT - 1))
                            nc.tensor.matmul(rs_ps[h][0:1, :], ones16[:],
                                             pbs[half][:, col],
                                             start=(kt == 0),
                                             stop=(kt == KT - 1))
                return o_ps, rs_ps

            pending = None
            for qc in range(QC):
                qs = slice(qc * QCW, (qc + 1) * QCW)
                HB = KT // NB  # k-tiles per batch
                bts = []
                pbs = []
                for half in range(NB):
                    bt = work.tile([128, HB * 1024], f16, tag="batch",
                                   bufs=4, name=f"ssqb_{qc}_{half}")
                    bts.append(bt)
                    for k8 in range(HB):
                        kt = half * HB + k8
                        ks = slice(kt * 128, (kt + 1) * 128)
                        s_re = psA.tile([128, 1024], f32, tag="s_re",
                                        name=f"s_re_{qc}_{kt}")
                        s_im = psA.tile([128, 1024], f32, tag="s_im",
                                        name=f"s_im_{qc}_{kt}")
                        for h in (0, 1):
                            col = slice(h * 512, h * 512 + 512)
                            nc.tensor.matmul(s_re[:, col], A_sb[h][:, ks],
                                             q_sb[h][:, qs],
                                             start=True, stop=True)
                            nc.tensor.matmul(s_im[:, col], C_sb[h][:, ks],
                                             q_sb[h][:, qs],
                                             start=True, stop=True)
                        t16 = work.tile([128, 1024], f16, tag="t16")
                        nc.vector.tensor_copy(t16[:], s_re[:])
                        sqre = work.tile([128, 1024], f16, tag="sqre")
                        nc.vector.tensor_mul(sqre[:], t16[:], t16[:])
                        sqim = work.tile([128, 1024], f16, tag="sqim")
                        if kt % 3 == 1:
                            ti16 = work.tile([128, 1024], f16, tag="ti16")
                            nc.vector.tensor_copy(ti16[:], s_im[:])
                            nc.vector.tensor_mul(sqim[:], ti16[:], ti16[:])
                        else:
                            nc.scalar.square(sqim[:], s_im[:])
                        nc.gpsimd.tensor_tensor(
                            bt[:, k8 * 1024:(k8 + 1) * 1024],
                            sqre[:], sqim[:], ALU.add)
                    # sqrt of this half right away (Square stays legal in
                    # sqrt_and_others, so no extra table traffic)
                    if half == 0:
                        load_act_set(SQRT_SET)
                    pb = work.tile([128, HB * 1024], f16, tag="batch",
                                   bufs=4, name=f"pb_{qc}_{half}")
                    pbs.append(pb)
                    nc.scalar.activation(pb[:], bt[:], AF.Sqrt,
                                         scale=1.0 / 64.0)
                load_act_set(EXP_SET)
                for pb in pbs:
                    nc.scalar.activation(pb[:], pb[:], AF.Exp,
                                         bias=eb_exp[:])
                if DEBUG and qc == 0:
                    dp = pers.tile([128, 1024], f32, tag="dbg_p00",
                                   name="dbg_p00")
                    nc.vector.tensor_copy(dp[:], pbs[0][:, 0:1024])
                    nc.sync.dma_start(dbg_es["d_p00"][:], dp[:])
                if pending is not None:
                    ppbs, pqs, pqc = pending
                    o_ps, rs_ps = emit_av(ppbs, pqs, pqc)
                    emit_norm(o_ps, rs_ps, pqs, pqc)
                    emit_wo(pqs, pqc)
                pending = (pbs, qs, qc)
            ppbs, pqs, pqc = pending
            o_ps, rs_ps = emit_av(ppbs, pqs, pqc)
            emit_norm(o_ps, rs_ps, pqs, pqc)
            emit_wo(pqs, pqc)

            if DEBUG:
                for nm, t in (("d_qT_re", qT_re), ("d_kT_re", kT_re),
                              ("d_kT_imn", kT_imn), ("d_vT16_h0", vT16_h[0]),
                              ("d_v16_h0", v16_h[0]), ("d_oT_re", oT_re)):
                    dd = pers.tile(list(t.shape), f32, tag=f"dbg_{nm}",
                                   name=f"dbg_{nm}")
                    nc.vector.tensor_copy(dd[:], t[:])
                    nc.sync.dma_start(dbg_es[nm][:], dd[:])

    nc.finalize()
    return nc


def _get_nc():
    if "nc" not in _CACHE:
        _CACHE["nc"] = _build_nc()
    return _CACHE["nc"]


def _core_inputs(c, inputs):
    b = c // 4
    h0 = 2 * (c % 4)
    hs = slice(h0 * 64, h0 * 64 + 128)

    xpack = np.empty((NCC, 24, 128, NCW), np.float16)
    for t, name in enumerate(
            ("Q_real", "Q_imag", "K_real", "K_imag", "V_real", "V_imag")):
        xT = np.ascontiguousarray(inputs[name][b].T)          # (512, 2048)
        xpack[:, t * 4:(t + 1) * 4] = (
            xT.reshape(4, 128, NCC, NCW).transpose(2, 0, 1, 3))

    wlist = []
    for kind in ("q", "A", "C", "v"):
        base_r = inputs[{"q": "wq_r", "A": "wk_r", "C": "wk_r",
                         "v": "wv_r"}[kind]]
        base_i = inputs[{"q": "wq_i", "A": "wk_i", "C": "wk_i",
                         "v": "wv_i"}[kind]]
        for hh in (0, 1):
            rows = slice((h0 + hh) * 64, (h0 + hh) * 64 + 64)
            wr, wi_ = base_r[rows], base_i[rows]
            if kind == "C":
                # rows = [-k_i; k_r]
                w1 = np.vstack([-wi_, wr])       # x_re weights
                w2 = np.vstack([-wr, -wi_])      # x_im weights
            else:
                # rows = [p_r; p_i]
                w1 = np.vstack([wr, wi_])
                w2 = np.vstack([-wi_, wr])
            wlist += [w1, w2]
    arr = np.empty((64, 128, 128), np.float16)
    for wi, mat in enumerate(wlist):
        arr[wi * 4:(wi + 1) * 4] = np.ascontiguousarray(mat.T).reshape(
            4, 128, 128)
    wpack = np.ascontiguousarray(arr.transpose(1, 0, 2)).reshape(
        128, 64 * 128)

    wo_r_T = np.ascontiguousarray(inputs["wo_r"][:, hs].T)    # (128, 512)
    wo_i_T = np.ascontiguousarray(inputs["wo_i"][:, hs].T)
    wopack = np.concatenate([wo_r_T, wo_i_T, -wo_i_T], axis=1)
    wopack = np.ascontiguousarray(wopack).astype(np.float16)

    return {
        "xpack": xpack,
        "wpack": wpack,
        "wopack": wopack,
        "onesr": np.ones((1, 128), np.float32),
        "ident": np.eye(128, dtype=np.float16),
    }


def kernel(**inputs):
    from concourse.bass_utils import run_bass_kernel_spmd

    nc = _get_nc()
    in_maps = [_core_inputs(c, inputs) for c in range(NCORES)]
    res = run_bass_kernel_spmd(nc, in_maps, list(range(NCORES)))
    out = np.empty((B, NQ, R, 2), np.float32)
    for b in range(B):
        re = np.zeros((512, NQ), np.float64)
        im = np.zeros((512, NQ), np.float64)
        for c in range(b * 4, b * 4 + 4):
            re += res.results[c]["out_re"]
            im += res.results[c]["out_im"]
        out[b, :, :, 0] = re.T
        out[b, :, :, 1] = im.T
    return out



# revision 9
# speedup vs baseline: 1.2777x; 1.2777x over previous
"""Complex-valued multi-head attention on 8 Trainium2 NeuronCores.

Sharding: batch(2) x head-pairs(4) -> 8 cores; each core runs one batch
element and 2 heads end-to-end (QKV proj -> complex scores -> |s| softmax
-> AV -> partial W_O), host sums the W_O partials over the 4 cores of each
batch element (tensor-parallel reduce) and transposes to the output layout.

Engine plan (cost-model driven, verifier-safe: at most one PSUM input per
instruction, no GPSIMD access to PSUM, one open PSUM accumulation group
per bank):
- PE: projections emit q/A stacked tensors ([p_r;p_i] x n) and V directly
  in [n, dv] layout; C = [-k_i; k_r] comes from partition-shifted
  negate/copy reads of the A projection PSUM (no extra matmuls).
- Scores stay [k, q]. |s|^2: s_im is extracted+squared by ACT Square for
  some k-tiles; the rest are DVE-copied to fp16 and squared on GPSIMD,
  which also does all the adds (SBUF-only). sqrt+exp run as one
  full-q-chunk ACT pass each, in place.
- AV is p-stationary per (head, 128-q block): one contiguous 16-matmul
  accumulation into [q, dv|rowsum] PSUM using a ones-column padded V, so
  the softmax denominator is column 128 of the same group. Normalization
  is a per-partition tensor_scalar; o is PE-transposed back to [dv, q]
  for W_O. Projection chunks 2..7 are interleaved into qc0's score loop.
"""
import sys

sys.path.insert(0, "/opt/trn_rl_repo")

import numpy as np

B, NQ, NK, R = 2, 2048, 2048, 512
H, DK, DV = 8, 64, 64
NCORES = 8
NCC = 8          # n-chunks for projection streaming (2048/256)
NCW = 256        # projection n-chunk width
QC = 4           # q-chunks in attention (2048/512)
QCW = 512
KT = 16          # k-tiles (2048/128)
VW = 129         # v16 block width (128 dv + ones column)

_CACHE = {}


def _build_nc():
    import concourse.bass as bass
    import concourse.tile as tile
    from concourse import bacc, mybir

    f32 = mybir.dt.float32
    f16 = mybir.dt.float16
    ALU = mybir.AluOpType
    AF = mybir.ActivationFunctionType

    nc = bacc.Bacc("TRN2", target_bir_lowering=False, debug=False,
                   num_devices=NCORES)

    xpack_e = nc.dram_tensor("xpack", [NCC, 128, 24 * NCW], f16,
                             kind="ExternalInput")
    wpack_e = nc.dram_tensor("wpack", [128, 48 * 128], f16,
                             kind="ExternalInput")
    wopack_e = nc.dram_tensor("wopack", [128, 4 * 512], f16,
                              kind="ExternalInput")
    ident_e = nc.dram_tensor("ident", [128, 128], f16, kind="ExternalInput")
    ore_e = nc.dram_tensor("out_re", [512, NQ], f16, kind="ExternalOutput")
    oim_e = nc.dram_tensor("out_im", [512, NQ], f16, kind="ExternalOutput")

    def ecopy(eng, dst, src):
        if eng == "act":
            nc.scalar.copy(dst, src)
        else:
            nc.vector.tensor_copy(dst, src)

    with tile.TileContext(nc) as tc:
      with nc.allow_low_precision(reason="fp16 softmax path"):
        with tc.tile_pool(name="pers", bufs=1) as pers, \
             tc.tile_pool(name="work", bufs=2) as work, \
             tc.tile_pool(name="psA", bufs=1, space="PSUM") as psA:

            # ---- constants ----
            wp = pers.tile([128, 48 * 128], f16, tag="wp")
            wop = pers.tile([128, 4 * 512], f16, tag="wop")
            ident16 = pers.tile([128, 128], f16, tag="ident16")
            eb_exp = pers.tile([128, 1], f32, tag="eb_exp")
            nc.vector.memset(eb_exp[:], -1.5)          # exp(mag - 1.5)

            q_sb = [pers.tile([128, NQ], f16, tag=f"q_sb{h}",
                              name=f"q_sb{h}") for h in (0, 1)]
            A_sb = [pers.tile([128, NK], f16, tag=f"A_sb{h}",
                              name=f"A_sb{h}") for h in (0, 1)]
            C_sb = [pers.tile([128, NK], f16, tag=f"C_sb{h}",
                              name=f"C_sb{h}") for h in (0, 1)]
            # V in [n, dv] blocks of width 129; col 128 of each block = 1.0
            v16_h = [pers.tile([128, KT * VW], f16, tag=f"v16_h{h}",
                               name=f"v16_h{h}") for h in (0, 1)]
            nc.gpsimd.memset(v16_h[0][:], 1.0)
            nc.gpsimd.memset(v16_h[1][:], 1.0)
            oT_h = [pers.tile([128, NQ], f16, tag=f"oT_h{h}",
                              name=f"oT_h{h}") for h in (0, 1)]

            # weight/const loads: first q-spec weights, then xt0 piece 0
            # (issued inside emit_proj(0)), then the rest
            nc.sync.dma_start(wp[:, 0:2048], wpack_e[:, 0:2048])

            def wblk(si, p):
                return wp[:, (si * 8 + p) * 128:(si * 8 + p + 1) * 128]

            # layout-A specs: (dest, tensor-pair, psum tag, copy engine)
            specsA = [(q_sb[0], 0, "s_re", "act"), (q_sb[1], 0, "s_im", "dve"),
                      (A_sb[0], 1, "s_re", "act"), (A_sb[1], 1, "s_im", "dve")]

            def emit_proj(ncc):
                if ncc == 0:
                    xt = work.tile([128, 24 * NCW], f16, tag="xt")
                    nc.sync.dma_start(xt[:, 0:2048], xpack_e[0][:, 0:2048])
                    nc.sync.dma_start(wp[:, 2048:6144], wpack_e[:, 2048:6144])
                    nc.sync.dma_start(xt[:, 2048:4096],
                                      xpack_e[0][:, 2048:4096])
                    nc.sync.dma_start(xt[:, 4096:6144],
                                      xpack_e[0][:, 4096:6144])
                    nc.sync.dma_start(wop[:], wopack_e[:])
                    nc.sync.dma_start(ident16[:], ident_e[:])
                else:
                    xt = work.tile([128, 24 * NCW], f16, tag="xt")
                    nc.sync.dma_start(xt[:], xpack_e[ncc])
                cs = slice(ncc * NCW, (ncc + 1) * NCW)

                def xblk(slot, c0, cw):
                    return xt[:, slot * NCW + c0:slot * NCW + c0 + cw]

                for si, (dest, tp, ptag, ceng) in enumerate(specsA):
                    pj = psA.tile([128, 1024], f32, tag=ptag,
                                  name=f"pj_{ncc}_{si}")
                    for rc in range(4):
                        nc.tensor.matmul(pj[:, 0:NCW], wblk(si, rc),
                                         xblk(2 * tp * 4 + rc, 0, NCW),
                                         start=(rc == 0), stop=False)
                    for rc in range(4):
                        nc.tensor.matmul(pj[:, 0:NCW], wblk(si, 4 + rc),
                                         xblk((2 * tp + 1) * 4 + rc, 0, NCW),
                                         start=False, stop=(rc == 3))
                    ecopy(ceng, dest[:, cs], pj[:, 0:NCW])
                    if si >= 2:
                        # C = [-k_i; k_r] via partition-shifted psum reads
                        h = si - 2
                        e2 = "act" if h == 0 else "dve"
                        if h == 0:
                            nc.scalar.mul(C_sb[h][0:64, cs],
                                          pj[64:128, 0:NCW], -1.0)
                        else:
                            nc.vector.tensor_scalar(
                                C_sb[h][0:64, cs], pj[64:128, 0:NCW],
                                -1.0, None, ALU.mult)
                        ecopy(e2, C_sb[h][64:128, cs], pj[0:64, 0:NCW])

                # layout-B V projection: [n, dv] blocks
                for h in (0, 1):
                    si = 4 + h
                    for nb in (0, 1):
                        vps = psA.tile([128, VW], f32, tag="oacc", bufs=3,
                                       name=f"vps_{ncc}_{h}_{nb}")
                        for rc in range(4):
                            nc.tensor.matmul(vps[:, 0:128],
                                             xblk(16 + rc, nb * 128, 128),
                                             wblk(si, rc),
                                             start=(rc == 0), stop=False)
                        for rc in range(4):
                            nc.tensor.matmul(vps[:, 0:128],
                                             xblk(20 + rc, nb * 128, 128),
                                             wblk(si, 4 + rc),
                                             start=False, stop=(rc == 3))
                        nt = 2 * ncc + nb
                        ecopy("act" if h == 0 else "dve",
                              v16_h[h][:, nt * VW:nt * VW + 128],
                              vps[:, 0:128])

            emit_proj(0)
            emit_proj(1)

            # ---- attention ----
            def emit_av_group(pend, g):
                h, qb = g // 4, g % 4
                pbt = pend["bt"]
                oacc = psA.tile([128, VW], f32, tag="oacc", bufs=3,
                                name=f"oacc_{pend['qc']}_{g}")
                pend["oacc"][g] = oacc
                for kt in range(KT):
                    stat = pbt[:, kt * 1024 + h * 512 + qb * 128:
                               kt * 1024 + h * 512 + qb * 128 + 128]
                    nc.tensor.matmul(oacc[:], stat,
                                     v16_h[h][:, kt * VW:(kt + 1) * VW],
                                     start=(kt == 0), stop=(kt == KT - 1))

            def emit_post(pend, g):
                h, qb = g // 4, g % 4
                oacc, pqs, pqc = pend["oacc"][g], pend["qs"], pend["qc"]
                recip = work.tile([128, 1], f32, tag="recip", bufs=2,
                                  name=f"recip_{pqc}_{g}")
                nc.vector.reciprocal(recip[:], oacc[:, 128:129])
                o_sb = work.tile([128, 128], f16, tag="osb", bufs=2,
                                 name=f"osb_{pqc}_{g}")
                nc.vector.tensor_scalar(o_sb[:], oacc[:, 0:128],
                                        recip[:], None, ALU.mult)
                otp = psA.tile([128, 256], f16, tag="otps",
                               name=f"otp_{pqc}_{g}")
                dst = otp[:, (g % 2) * 128:(g % 2) * 128 + 128]
                nc.tensor.transpose(dst, o_sb[:], ident16[:])
                nc.vector.tensor_copy(
                    oT_h[h][:, pqs.start + qb * 128:
                            pqs.start + qb * 128 + 128], dst)

            def emit_wo(pend):
                pqs, pqc = pend["qs"], pend["qc"]

                def wob(hh, out, Rc):
                    base = (hh * 2 + out) * 512 + Rc * 128
                    return wop[:, base:base + 128]

                for half_rc in (0, 1):
                    wo_re = psA.tile([128, 1024], f32, tag="s_re",
                                     name=f"wore_{pqc}_{half_rc}")
                    wo_im = psA.tile([128, 1024], f32, tag="s_im",
                                     name=f"woim_{pqc}_{half_rc}")
                    for i in (0, 1):
                        Rc = half_rc * 2 + i
                        cs = slice(i * 512, (i + 1) * 512)
                        nc.tensor.matmul(wo_re[:, cs], wob(0, 0, Rc),
                                         oT_h[0][:, pqs],
                                         start=True, stop=False)
                        nc.tensor.matmul(wo_re[:, cs], wob(1, 0, Rc),
                                         oT_h[1][:, pqs],
                                         start=False, stop=True)
                        nc.tensor.matmul(wo_im[:, cs], wob(0, 1, Rc),
                                         oT_h[0][:, pqs],
                                         start=True, stop=False)
                        nc.tensor.matmul(wo_im[:, cs], wob(1, 1, Rc),
                                         oT_h[1][:, pqs],
                                         start=False, stop=True)
                    st_re = work.tile([128, 1024], f16, tag="st_re",
                                      name=f"stre_{pqc}_{half_rc}")
                    nc.vector.tensor_copy(st_re[:], wo_re[:])
                    st_im = work.tile([128, 1024], f16, tag="st_im",
                                      name=f"stim_{pqc}_{half_rc}")
                    nc.vector.tensor_copy(st_im[:], wo_im[:])
                    for i in (0, 1):
                        Rc = half_rc * 2 + i
                        cs = slice(i * 512, (i + 1) * 512)
                        nc.sync.dma_start(
                            ore_e[Rc * 128:(Rc + 1) * 128, pqs], st_re[:, cs])
                        nc.sync.dma_start(
                            oim_e[Rc * 128:(Rc + 1) * 128, pqs], st_im[:, cs])

            ACT_IM = {0, 4, 8, 12}  # im-square on ACT for these k-tiles

            def emit_tr_piece(tbt, piece, npc=16):
                # sqrt/exp of the previous chunk's bt, in 2*npc pieces:
                # pieces 0..npc-1 sqrt, npc..2*npc-1 exp
                pw = KT * 1024 // npc
                if piece < npc:
                    sl = slice(piece * pw, (piece + 1) * pw)
                    nc.scalar.activation(tbt[:, sl], tbt[:, sl], AF.Sqrt,
                                         scale=1.0 / 64.0)
                else:
                    sl = slice((piece - npc) * pw, (piece - npc + 1) * pw)
                    nc.scalar.activation(tbt[:, sl], tbt[:, sl], AF.Exp,
                                         bias=eb_exp[:])

            pend_q = []
            tr_pend = None
            for qc in range(QC):
                qs = slice(qc * QCW, (qc + 1) * QCW)
                bt = work.tile([128, KT * 1024], f16, tag="bt", bufs=3,
                               name=f"bt_{qc}")
                # process the 2-chunks-old pending (its exp is long done)
                pend = pend_q.pop(0) if len(pend_q) == 2 else None
                for kt in range(KT):
                    ks = slice(kt * 128, (kt + 1) * 128)
                    s_re = psA.tile([128, 1024], f32, tag="s_re",
                                    name=f"s_re_{qc}_{kt}")
                    s_im = psA.tile([128, 1024], f32, tag="s_im",
                                    name=f"s_im_{qc}_{kt}")
                    for h in (0, 1):
                        col = slice(h * 512, h * 512 + 512)
                        nc.tensor.matmul(s_re[:, col], A_sb[h][:, ks],
                                         q_sb[h][:, qs],
                                         start=True, stop=True)
                        nc.tensor.matmul(s_im[:, col], C_sb[h][:, ks],
                                         q_sb[h][:, qs],
                                         start=True, stop=True)
                    if qc == 0 and kt % 2 == 1 and kt <= 11:
                        emit_proj(2 + (kt - 1) // 2)
                    if pend is not None and kt % 2 == 1:
                        emit_av_group(pend, kt // 2)
                        if kt >= 5:
                            emit_post(pend, (kt - 5) // 2)
                    # |s|^2 extraction
                    t_re = work.tile([128, 1024], f16, tag="tre", bufs=2,
                                     name=f"tre_{qc}_{kt}")
                    nc.vector.tensor_copy(t_re[:], s_re[:])
                    sq_re = work.tile([128, 1024], f16, tag="sqre", bufs=2,
                                      name=f"sqre_{qc}_{kt}")
                    nc.gpsimd.tensor_tensor(sq_re[:], t_re[:], t_re[:],
                                            ALU.mult)
                    if kt in ACT_IM:
                        sq_im = work.tile([128, 1024], f16, tag="sqim",
                                          bufs=2, name=f"sqim_{qc}_{kt}")
                        nc.scalar.square(sq_im[:], s_im[:])
                    else:
                        t_im = work.tile([128, 1024], f16, tag="tim", bufs=2,
                                         name=f"tim_{qc}_{kt}")
                        nc.vector.tensor_copy(t_im[:], s_im[:])
                        sq_im = work.tile([128, 1024], f16, tag="sqim",
                                          bufs=2, name=f"sqim_{qc}_{kt}")
                        nc.gpsimd.tensor_tensor(sq_im[:], t_im[:], t_im[:],
                                                ALU.mult)
                    nc.gpsimd.tensor_tensor(bt[:, kt * 1024:(kt + 1) * 1024],
                                            sq_re[:], sq_im[:], ALU.add)
                    if tr_pend is not None:
                        emit_tr_piece(tr_pend, kt, npc=8)
                if pend is not None:
                    emit_post(pend, 6)
                    emit_post(pend, 7)
                    emit_wo(pend)
                pend_q.append({"bt": bt, "qs": qs, "qc": qc,
                               "oacc": [None] * 8})
                tr_pend = bt

            # tail: transcendentals for the last chunk, then flush AV of the
            # last two chunks (qc2's exp finished during qc3's window)
            for piece in range(8):
                emit_tr_piece(tr_pend, piece, npc=4)
            for pend in pend_q:
                for g in range(8):
                    emit_av_group(pend, g)
                    if g >= 2:
                        emit_post(pend, g - 2)
                emit_post(pend, 6)
                emit_post(pend, 7)
                emit_wo(pend)

    nc.finalize()
    return nc


def _get_nc():
    if "nc" not in _CACHE:
        _CACHE["nc"] = _build_nc()
    return _CACHE["nc"]


def _core_inputs(c, inputs):
    b = c // 4
    h0 = 2 * (c % 4)

    # xpack[ncc, p, s*NCW + f] = xT[rc*128 + p, ncc*NCW + f], s = t*4 + rc
    xpack = np.empty((NCC, 128, 24, NCW), np.float16)
    for t, name in enumerate(
            ("Q_real", "Q_imag", "K_real", "K_imag", "V_real", "V_imag")):
        xT = np.ascontiguousarray(inputs[name][b].T)          # (512, 2048)
        blk = xT.reshape(4, 128, NCC, NCW)                    # (rc, p, ncc, f)
        xpack[:, :, t * 4:(t + 1) * 4, :] = blk.transpose(2, 1, 0, 3)
    xpack = xpack.reshape(NCC, 128, 24 * NCW)

    # wpack: 6 specs (q_h0, q_h1, A_h0, A_h1, v_h0, v_h1) x 8 pass-blocks
    blocks = []
    for nr, ni in (("wq_r", "wq_i"), ("wk_r", "wk_i"), ("wv_r", "wv_i")):
        for hh in (0, 1):
            rows = slice((h0 + hh) * 64, (h0 + hh) * 64 + 64)
            wr = inputs[nr][rows].astype(np.float32)
            wi = inputs[ni][rows].astype(np.float32)
            w1 = np.vstack([wr, wi]).T       # (512, 128) for x_re passes
            w2 = np.vstack([-wi, wr]).T      # (512, 128) for x_im passes
            for rc in range(4):
                blocks.append(w1[rc * 128:(rc + 1) * 128])
            for rc in range(4):
                blocks.append(w2[rc * 128:(rc + 1) * 128])
    wpack = np.concatenate(blocks, axis=1).astype(np.float16)

    # wopack: per (hh, out) one [128, 512] stationary strip
    wo_blocks = []
    for hh in (0, 1):
        hsl = slice((h0 + hh) * 64, (h0 + hh) * 64 + 64)
        wr = inputs["wo_r"][:, hsl].astype(np.float32)        # (512, 64)
        wi = inputs["wo_i"][:, hsl].astype(np.float32)
        wo_blocks.append(np.vstack([wr.T, -wi.T]))            # out_re
        wo_blocks.append(np.vstack([wi.T, wr.T]))             # out_im
    wopack = np.concatenate(wo_blocks, axis=1).astype(np.float16)

    return {
        "xpack": np.ascontiguousarray(xpack),
        "wpack": np.ascontiguousarray(wpack),
        "wopack": np.ascontiguousarray(wopack),
        "ident": np.eye(128, dtype=np.float16),
    }


def kernel(**inputs):
    from concourse.bass_utils import run_bass_kernel_spmd

    nc = _get_nc()
    in_maps = [_core_inputs(c, inputs) for c in range(NCORES)]
    res = run_bass_kernel_spmd(nc, in_maps, list(range(NCORES)))
    out = np.empty((B, NQ, R, 2), np.float32)
    for b in range(B):
        re = np.zeros((512, NQ), np.float32)
        im = np.zeros((512, NQ), np.float32)
        for c in range(b * 4, b * 4 + 4):
            re += res.results[c]["out_re"].astype(np.float32)
            im += res.results[c]["out_im"].astype(np.float32)
        out[b, :, :, 0] = re.T
        out[b, :, :, 1] = im.T
    return out


# revision 17
# speedup vs baseline: 1.3201x; 1.0332x over previous
"""Complex-valued multi-head attention on 8 Trainium2 NeuronCores.

Sharding: batch(2) x head-pairs(4) -> 8 cores; each core runs one batch
element and 2 heads end-to-end (QKV proj -> complex scores -> |s| softmax
-> AV -> partial W_O), host sums the W_O partials over the 4 cores of each
batch element (tensor-parallel reduce) and transposes to the output layout.

Engine plan (cost-model driven, verifier-safe: at most one PSUM input per
instruction, no GPSIMD access to PSUM, one open PSUM accumulation group
per bank):
- PE: projections emit q/A stacked tensors ([p_r;p_i] x n) and V directly
  in [n, dv] layout; C = [-k_i; k_r] comes from partition-shifted
  negate/copy reads of the A projection PSUM (no extra matmuls).
- Scores stay [k, q]. |s|^2: s_im is extracted+squared by ACT Square for
  some k-tiles; the rest are DVE-copied to fp16 and squared on GPSIMD,
  which also does all the adds (SBUF-only). sqrt+exp run as one
  full-q-chunk ACT pass each, in place.
- AV is p-stationary per (head, 128-q block): one contiguous 16-matmul
  accumulation into [q, dv|rowsum] PSUM using a ones-column padded V, so
  the softmax denominator is column 128 of the same group. Normalization
  is a per-partition tensor_scalar; o is PE-transposed back to [dv, q]
  for W_O. Projection chunks 2..7 are interleaved into qc0's score loop.
"""
import sys

sys.path.insert(0, "/opt/trn_rl_repo")

import numpy as np

B, NQ, NK, R = 2, 2048, 2048, 512
H, DK, DV = 8, 64, 64
NCORES = 8
NCC = 8          # n-chunks for projection streaming (2048/256)
NCW = 256        # projection n-chunk width
QC = 4           # q-chunks in attention (2048/512)
QCW = 512
KT = 16          # k-tiles (2048/128)
VW = 129         # v16 block width (128 dv + ones column)

_CACHE = {}


def _build_nc():
    import concourse.bass as bass
    import concourse.tile as tile
    from concourse import bacc, mybir

    f32 = mybir.dt.float32
    f16 = mybir.dt.float16
    ALU = mybir.AluOpType
    AF = mybir.ActivationFunctionType

    nc = bacc.Bacc("TRN2", target_bir_lowering=False, debug=False,
                   num_devices=NCORES)

    xpack_e = nc.dram_tensor("xpack", [NCC, 128, 24 * NCW], f16,
                             kind="ExternalInput")
    wpack_e = nc.dram_tensor("wpack", [128, 48 * 128], f16,
                             kind="ExternalInput")
    wopack_e = nc.dram_tensor("wopack", [128, 4 * 512], f16,
                              kind="ExternalInput")
    ident_e = nc.dram_tensor("ident", [128, 128], f16, kind="ExternalInput")
    ore_e = nc.dram_tensor("out_re", [512, NQ], f16, kind="ExternalOutput")
    oim_e = nc.dram_tensor("out_im", [512, NQ], f16, kind="ExternalOutput")

    def ecopy(eng, dst, src):
        if eng == "act":
            nc.scalar.copy(dst, src)
        else:
            nc.vector.tensor_copy(dst, src)

    with tile.TileContext(nc) as tc:
      with nc.allow_low_precision(reason="fp16 softmax path"):
        with tc.tile_pool(name="pers", bufs=1) as pers, \
             tc.tile_pool(name="work", bufs=2) as work, \
             tc.tile_pool(name="psA", bufs=1, space="PSUM") as psA:

            # ---- constants ----
            wp = pers.tile([128, 48 * 128], f16, tag="wp")
            wop = pers.tile([128, 4 * 512], f16, tag="wop")
            ident16 = pers.tile([128, 128], f16, tag="ident16")
            eb_exp = pers.tile([128, 1], f32, tag="eb_exp")
            nc.vector.memset(eb_exp[:], -1.5)          # exp(mag - 1.5)

            q_sb = [pers.tile([128, NQ], f16, tag=f"q_sb{h}",
                              name=f"q_sb{h}") for h in (0, 1)]
            A_sb = [pers.tile([128, NK], f16, tag=f"A_sb{h}",
                              name=f"A_sb{h}") for h in (0, 1)]
            C_sb = [pers.tile([128, NK], f16, tag=f"C_sb{h}",
                              name=f"C_sb{h}") for h in (0, 1)]
            # V in [n, dv] blocks of width 129; col 128 of each block = 1.0
            v16_h = [pers.tile([128, KT * VW], f16, tag=f"v16_h{h}",
                               name=f"v16_h{h}") for h in (0, 1)]
            nc.gpsimd.memset(v16_h[0][:], 1.0)
            nc.gpsimd.memset(v16_h[1][:], 1.0)
            oT_h = [pers.tile([128, NQ], f16, tag=f"oT_h{h}",
                              name=f"oT_h{h}") for h in (0, 1)]

            # weight/const loads: first q-spec weights, then xt0 piece 0
            # (issued inside emit_proj(0)), then the rest
            nc.sync.dma_start(wp[:, 0:2048], wpack_e[:, 0:2048])

            def wblk(si, p):
                return wp[:, (si * 8 + p) * 128:(si * 8 + p + 1) * 128]

            # layout-A specs: (dest, tensor-pair, psum tag, copy engine)
            specsA = [(q_sb[0], 0, "s_re", "act"), (q_sb[1], 0, "s_im", "dve"),
                      (A_sb[0], 1, "s_re", "act"), (A_sb[1], 1, "s_im", "dve")]

            def emit_proj(ncc):
                if ncc == 0:
                    xt = work.tile([128, 24 * NCW], f16, tag="xt")
                    nc.sync.dma_start(xt[:, 0:2048], xpack_e[0][:, 0:2048])
                    nc.sync.dma_start(wp[:, 2048:6144], wpack_e[:, 2048:6144])
                    nc.sync.dma_start(xt[:, 2048:4096],
                                      xpack_e[0][:, 2048:4096])
                    nc.sync.dma_start(xt[:, 4096:6144],
                                      xpack_e[0][:, 4096:6144])
                    nc.sync.dma_start(wop[:], wopack_e[:])
                    nc.sync.dma_start(ident16[:], ident_e[:])
                else:
                    xt = work.tile([128, 24 * NCW], f16, tag="xt")
                    nc.sync.dma_start(xt[:], xpack_e[ncc])
                cs = slice(ncc * NCW, (ncc + 1) * NCW)

                def xblk(slot, c0, cw):
                    return xt[:, slot * NCW + c0:slot * NCW + c0 + cw]

                for si, (dest, tp, ptag, ceng) in enumerate(specsA):
                    pj = psA.tile([128, 1024], f32, tag=ptag,
                                  name=f"pj_{ncc}_{si}")
                    for rc in range(4):
                        nc.tensor.matmul(pj[:, 0:NCW], wblk(si, rc),
                                         xblk(2 * tp * 4 + rc, 0, NCW),
                                         start=(rc == 0), stop=False)
                    for rc in range(4):
                        nc.tensor.matmul(pj[:, 0:NCW], wblk(si, 4 + rc),
                                         xblk((2 * tp + 1) * 4 + rc, 0, NCW),
                                         start=False, stop=(rc == 3))
                    ecopy(ceng, dest[:, cs], pj[:, 0:NCW])
                    if si >= 2:
                        # C = [-k_i; k_r] via partition-shifted psum reads
                        h = si - 2
                        e2 = "act" if h == 0 else "dve"
                        if h == 0:
                            nc.scalar.mul(C_sb[h][0:64, cs],
                                          pj[64:128, 0:NCW], -1.0)
                        else:
                            nc.vector.tensor_scalar(
                                C_sb[h][0:64, cs], pj[64:128, 0:NCW],
                                -1.0, None, ALU.mult)
                        ecopy(e2, C_sb[h][64:128, cs], pj[0:64, 0:NCW])

                # layout-B V projection: [n, dv] blocks
                for h in (0, 1):
                    si = 4 + h
                    for nb in (0, 1):
                        vps = psA.tile([128, VW], f32, tag="oacc", bufs=3,
                                       name=f"vps_{ncc}_{h}_{nb}")
                        for rc in range(4):
                            nc.tensor.matmul(vps[:, 0:128],
                                             xblk(16 + rc, nb * 128, 128),
                                             wblk(si, rc),
                                             start=(rc == 0), stop=False)
                        for rc in range(4):
                            nc.tensor.matmul(vps[:, 0:128],
                                             xblk(20 + rc, nb * 128, 128),
                                             wblk(si, 4 + rc),
                                             start=False, stop=(rc == 3))
                        nt = 2 * ncc + nb
                        ecopy("act" if h == 0 else "dve",
                              v16_h[h][:, nt * VW:nt * VW + 128],
                              vps[:, 0:128])

            emit_proj(0)
            emit_proj(1)

            # ---- attention ----
            def emit_av_group(pend, g):
                h, qb = g // 4, g % 4
                pbt = pend["bt"]
                oacc = psA.tile([128, VW], f32, tag="oacc", bufs=3,
                                name=f"oacc_{pend['qc']}_{g}")
                pend["oacc"][g] = oacc
                for kt in range(KT):
                    stat = pbt[:, kt * 1024 + h * 512 + qb * 128:
                               kt * 1024 + h * 512 + qb * 128 + 128]
                    nc.tensor.matmul(oacc[:], stat,
                                     v16_h[h][:, kt * VW:(kt + 1) * VW],
                                     start=(kt == 0), stop=(kt == KT - 1))

            def emit_post(pend, g):
                h, qb = g // 4, g % 4
                oacc, pqs, pqc = pend["oacc"][g], pend["qs"], pend["qc"]
                recip = work.tile([128, 1], f32, tag="recip", bufs=2,
                                  name=f"recip_{pqc}_{g}")
                nc.vector.reciprocal(recip[:], oacc[:, 128:129])
                o_sb = work.tile([128, 128], f16, tag="osb", bufs=2,
                                 name=f"osb_{pqc}_{g}")
                nc.vector.tensor_scalar(o_sb[:], oacc[:, 0:128],
                                        recip[:], None, ALU.mult)
                otp = psA.tile([128, 256], f16, tag="otps",
                               name=f"otp_{pqc}_{g}")
                dst = otp[:, (g % 2) * 128:(g % 2) * 128 + 128]
                nc.tensor.transpose(dst, o_sb[:], ident16[:])
                nc.vector.tensor_copy(
                    oT_h[h][:, pqs.start + qb * 128:
                            pqs.start + qb * 128 + 128], dst)

            def emit_wo(pend):
                pqs, pqc = pend["qs"], pend["qc"]

                def wob(hh, out, Rc):
                    base = (hh * 2 + out) * 512 + Rc * 128
                    return wop[:, base:base + 128]

                for half_rc in (0, 1):
                    wo_re = psA.tile([128, 1024], f32, tag="s_re",
                                     name=f"wore_{pqc}_{half_rc}")
                    wo_im = psA.tile([128, 1024], f32, tag="s_im",
                                     name=f"woim_{pqc}_{half_rc}")
                    for i in (0, 1):
                        Rc = half_rc * 2 + i
                        cs = slice(i * 512, (i + 1) * 512)
                        nc.tensor.matmul(wo_re[:, cs], wob(0, 0, Rc),
                                         oT_h[0][:, pqs],
                                         start=True, stop=False)
                        nc.tensor.matmul(wo_re[:, cs], wob(1, 0, Rc),
                                         oT_h[1][:, pqs],
                                         start=False, stop=True)
                        nc.tensor.matmul(wo_im[:, cs], wob(0, 1, Rc),
                                         oT_h[0][:, pqs],
                                         start=True, stop=False)
                        nc.tensor.matmul(wo_im[:, cs], wob(1, 1, Rc),
                                         oT_h[1][:, pqs],
                                         start=False, stop=True)
                    st_re = work.tile([128, 1024], f16, tag="st_re",
                                      name=f"stre_{pqc}_{half_rc}")
                    nc.vector.tensor_copy(st_re[:], wo_re[:])
                    st_im = work.tile([128, 1024], f16, tag="st_im",
                                      name=f"stim_{pqc}_{half_rc}")
                    nc.vector.tensor_copy(st_im[:], wo_im[:])
                    for i in (0, 1):
                        Rc = half_rc * 2 + i
                        cs = slice(i * 512, (i + 1) * 512)
                        nc.sync.dma_start(
                            ore_e[Rc * 128:(Rc + 1) * 128, pqs], st_re[:, cs])
                        nc.sync.dma_start(
                            oim_e[Rc * 128:(Rc + 1) * 128, pqs], st_im[:, cs])

            ACT_IM = {0, 4, 8, 12}  # im-square on ACT for these k-tiles

            def emit_tr_piece(tbt, piece, npc=8):
                # sqrt/exp of an older chunk's bt in 2*npc pieces:
                # pieces 0..npc-1 sqrt, npc..2*npc-1 exp
                pw = KT * 1024 // npc
                if piece < npc:
                    sl = slice(piece * pw, (piece + 1) * pw)
                    nc.scalar.activation(tbt[:, sl], tbt[:, sl], AF.Sqrt,
                                         scale=1.0 / 64.0)
                else:
                    sl = slice((piece - npc) * pw, (piece - npc + 1) * pw)
                    nc.scalar.activation(tbt[:, sl], tbt[:, sl], AF.Exp,
                                         bias=eb_exp[:])

            pend_q = []
            tr_pend = None
            for qc in range(QC):
                qs = slice(qc * QCW, (qc + 1) * QCW)
                bt = work.tile([128, KT * 1024], f16, tag="bt", bufs=3,
                               name=f"bt_{qc}")
                # process the 2-chunks-old pending (its exp is long done)
                pend = pend_q.pop(0) if len(pend_q) == 2 else None
                for kt in range(KT):
                    ks = slice(kt * 128, (kt + 1) * 128)
                    s_re = psA.tile([128, 1024], f32, tag="s_re",
                                    name=f"s_re_{qc}_{kt}")
                    s_im = psA.tile([128, 1024], f32, tag="s_im",
                                    name=f"s_im_{qc}_{kt}")
                    for h in (0, 1):
                        col = slice(h * 512, h * 512 + 512)
                        nc.tensor.matmul(s_re[:, col], A_sb[h][:, ks],
                                         q_sb[h][:, qs],
                                         start=True, stop=True)
                        nc.tensor.matmul(s_im[:, col], C_sb[h][:, ks],
                                         q_sb[h][:, qs],
                                         start=True, stop=True)
                    if qc == 0 and kt % 2 == 1 and kt <= 11:
                        emit_proj(2 + (kt - 1) // 2)
                    if pend is not None and kt % 2 == 1:
                        emit_av_group(pend, kt // 2)
                        if kt >= 5:
                            emit_post(pend, (kt - 5) // 2)
                    # |s|^2 extraction
                    t_re = work.tile([128, 1024], f16, tag="tre", bufs=2,
                                     name=f"tre_{qc}_{kt}")
                    nc.vector.tensor_copy(t_re[:], s_re[:])
                    sq_re = work.tile([128, 1024], f16, tag="sqre", bufs=2,
                                      name=f"sqre_{qc}_{kt}")
                    nc.gpsimd.tensor_tensor(sq_re[:], t_re[:], t_re[:],
                                            ALU.mult)
                    if kt in ACT_IM:
                        sq_im = work.tile([128, 1024], f16, tag="sqim",
                                          bufs=2, name=f"sqim_{qc}_{kt}")
                        nc.scalar.square(sq_im[:], s_im[:])
                    else:
                        t_im = work.tile([128, 1024], f16, tag="tim", bufs=2,
                                         name=f"tim_{qc}_{kt}")
                        nc.vector.tensor_copy(t_im[:], s_im[:])
                        sq_im = work.tile([128, 1024], f16, tag="sqim",
                                          bufs=2, name=f"sqim_{qc}_{kt}")
                        nc.gpsimd.tensor_tensor(sq_im[:], t_im[:], t_im[:],
                                                ALU.mult)
                    nc.gpsimd.tensor_tensor(bt[:, kt * 1024:(kt + 1) * 1024],
                                            sq_re[:], sq_im[:], ALU.add)
                    if tr_pend is not None:
                        emit_tr_piece(tr_pend, kt, npc=8)
                if pend is not None:
                    emit_post(pend, 6)
                    emit_post(pend, 7)
                    emit_wo(pend)
                pend_q.append({"bt": bt, "qs": qs, "qc": qc,
                               "oacc": [None] * 8})
                tr_pend = bt

            # tail: transcendentals for the last chunk, then flush AV of the
            # last two chunks (qc2's exp finished during qc3's window)
            for piece in range(8):
                emit_tr_piece(tr_pend, piece, npc=4)
            for pend in pend_q:
                for g in range(8):
                    emit_av_group(pend, g)
                    if g >= 2:
                        emit_post(pend, g - 2)
                emit_post(pend, 6)
                emit_post(pend, 7)
                emit_wo(pend)

    nc.finalize()
    return nc


def _get_nc():
    if "nc" not in _CACHE:
        _CACHE["nc"] = _build_nc()
    return _CACHE["nc"]


def _core_inputs(c, inputs):
    b = c // 4
    h0 = 2 * (c % 4)

    # xpack[ncc, p, s*NCW + f] = xT[rc*128 + p, ncc*NCW + f], s = t*4 + rc
    xpack = np.empty((NCC, 128, 24, NCW), np.float16)
    for t, name in enumerate(
            ("Q_real", "Q_imag", "K_real", "K_imag", "V_real", "V_imag")):
        xT = np.ascontiguousarray(inputs[name][b].T)          # (512, 2048)
        blk = xT.reshape(4, 128, NCC, NCW)                    # (rc, p, ncc, f)
        xpack[:, :, t * 4:(t + 1) * 4, :] = blk.transpose(2, 1, 0, 3)
    xpack = xpack.reshape(NCC, 128, 24 * NCW)

    # wpack: 6 specs (q_h0, q_h1, A_h0, A_h1, v_h0, v_h1) x 8 pass-blocks
    blocks = []
    for nr, ni in (("wq_r", "wq_i"), ("wk_r", "wk_i"), ("wv_r", "wv_i")):
        for hh in (0, 1):
            rows = slice((h0 + hh) * 64, (h0 + hh) * 64 + 64)
            wr = inputs[nr][rows].astype(np.float32)
            wi = inputs[ni][rows].astype(np.float32)
            w1 = np.vstack([wr, wi]).T       # (512, 128) for x_re passes
            w2 = np.vstack([-wi, wr]).T      # (512, 128) for x_im passes
            for rc in range(4):
                blocks.append(w1[rc * 128:(rc + 1) * 128])
            for rc in range(4):
                blocks.append(w2[rc * 128:(rc + 1) * 128])
    wpack = np.concatenate(blocks, axis=1).astype(np.float16)

    # wopack: per (hh, out) one [128, 512] stationary strip
    wo_blocks = []
    for hh in (0, 1):
        hsl = slice((h0 + hh) * 64, (h0 + hh) * 64 + 64)
        wr = inputs["wo_r"][:, hsl].astype(np.float32)        # (512, 64)
        wi = inputs["wo_i"][:, hsl].astype(np.float32)
        wo_blocks.append(np.vstack([wr.T, -wi.T]))            # out_re
        wo_blocks.append(np.vstack([wi.T, wr.T]))             # out_im
    wopack = np.concatenate(wo_blocks, axis=1).astype(np.float16)

    return {
        "xpack": np.ascontiguousarray(xpack),
        "wpack": np.ascontiguousarray(wpack),
        "wopack": np.ascontiguousarray(wopack),
        "ident": np.eye(128, dtype=np.float16),
    }


def kernel(**inputs):
    from concourse.bass_utils import run_bass_kernel_spmd

    nc = _get_nc()
    in_maps = [_core_inputs(c, inputs) for c in range(NCORES)]
    res = run_bass_kernel_spmd(nc, in_maps, list(range(NCORES)))
    out = np.empty((B, NQ, R, 2), np.float32)
    for b in range(B):
        re = np.zeros((512, NQ), np.float32)
        im = np.zeros((512, NQ), np.float32)
        for c in range(b * 4, b * 4 + 4):
            re += res.results[c]["out_re"].astype(np.float32)
            im += res.results[c]["out_im"].astype(np.float32)
        out[b, :, :, 0] = re.T
        out[b, :, :, 1] = im.T
    return out
